# revision 18
# baseline (speedup 1.0000x reference)
"""Trainium2 Bass kernel for an 8-expert top-2 MoE block (T=2048, D=1024, H=4096).

Strategy (hidden-dim sharding, perfectly balanced):
  - Host computes the gate (router logits, top-2, softmax weights) and sorts
    the T*k (token, expert) pairs by expert.  The pair list is cut into
    single-expert groups of <=512 columns (long expert runs are split into
    near-equal parts), so every group maps to one PSUM bank.
  - Each of the 8 cores owns a 512-wide slice of the HIDDEN dim (4 chunks of
    128) of EVERY expert.  Each core runs all 4096 pairs through
      P1: hT[h, p] = relu(sum_d W1[d, h] X[d, p] + b1[h])   (its h-slice)
      P2: y[d, p] += sum_h W2[h, d] hT[h, p]                (partial over h)
    All 8 cores execute the identical instruction stream (~109.2us of
    back-to-back fp16 matmuls = the PE roofline for this sharding), so the
    max-over-cores time equals the balanced optimum regardless of routing.
  - Host sums the 8 partial outputs, applies the combine weights and the
    (once-only) b2 bias, and scatter-gathers back to (T, D).
  - Head: a burst of dummy matmuls right after the preamble pre-warms the
    PE HAM clock gate while the first DMAs are in flight; the first group's
    xg/W1 transfers are split fine-grained so the real stream starts early.
"""

import os
import sys

for p in ("/opt/trn_rl_repo",):
    if p not in sys.path and os.path.isdir(p):
        sys.path.insert(0, p)

# The kernel needs the axon-tunneled NeuronCores; don't let a stray
# JAX_PLATFORMS=cpu (set by some harnesses for the reference) hide them.
if "jax" not in sys.modules and os.environ.get("JAX_PLATFORMS", "") == "cpu":
    del os.environ["JAX_PLATFORMS"]

from contextlib import ExitStack

import numpy as np

import concourse.bass as bass
import concourse.bacc as bacc
import concourse.mybir as mybir
import concourse.tile as tile
from concourse.bass_utils import run_bass_kernel_spmd

T, D, H, E = 2048, 1024, 4096, 8
NCORES = 8
DC = D // 128          # 8 contraction/output chunks of the model dim
HCL = H // 128 // NCORES  # 4 hidden chunks of 128 per core
GMAX = 512             # max pair-columns per group (one PSUM bank)
WARMUP = 24            # dummy matmuls to pre-warm the PE clock gate while
                       # the first ~4.5MB of DMA primes the pipeline
TAILG = 64            # target size of the final (tail-shortening) group
F32 = mybir.dt.float32
FP16 = mybir.dt.float16

_prog_cache = {}


def _plan_groups(counts):
    """Cut the expert-sorted pair list into single-expert groups <= GMAX.

    Experts are ordered by descending load (max PE work per byte of weight
    DMA at the head), and the final group is kept small so the kernel tail
    (last psum -> epilogue -> store -> barrier) is short.
    """
    order = sorted(range(len(counts)), key=lambda e: -counts[e])
    groups = []  # (expert, size)
    for e in order:
        n = int(counts[e])
        if n == 0:
            continue
        k = -(-n // GMAX)
        b, r = divmod(n, k)
        for i in range(k):
            groups.append((int(e), b + (1 if i < r else 0)))
    if groups and groups[-1][1] > TAILG + 32:
        e, n = groups[-1]
        groups[-1] = (e, n - TAILG)
        groups.append((e, TAILG))
    return tuple(groups)


def _build_program(groups):
    P = sum(s for _, s in groups)
    G = len(groups)
    nc = bacc.Bacc("TRN2", target_bir_lowering=False, debug=False)

    xg_d = nc.dram_tensor("xg", [128, DC * P], FP16, kind="ExternalInput")
    w1_d = nc.dram_tensor("w1", [E, 128, HCL, DC, 128], FP16, kind="ExternalInput")
    w2_d = nc.dram_tensor("w2", [E, 128, HCL, DC, 128], FP16, kind="ExternalInput")
    b1_d = nc.dram_tensor("b1", [128, E * HCL], F32, kind="ExternalInput")
    out_d = nc.dram_tensor("out", [128, DC * P], FP16, kind="ExternalOutput")
    junk_d = nc.dram_tensor("junk", [128, 4], F32, kind="ExternalOutput")

    with tile.TileContext(nc) as tc, ExitStack() as ctx:
        const = ctx.enter_context(tc.tile_pool(name="const", bufs=1))
        xgp = ctx.enter_context(tc.tile_pool(name="xgp", bufs=4))
        w1p = ctx.enter_context(tc.tile_pool(name="w1p", bufs=3))
        w2p = ctx.enter_context(tc.tile_pool(name="w2p", bufs=3))
        htp = ctx.enter_context(tc.tile_pool(name="htp", bufs=4))
        outp = ctx.enter_context(tc.tile_pool(name="outp", bufs=3))
        psp = ctx.enter_context(tc.tile_pool(name="psp", bufs=7, space="PSUM"))
        wps = ctx.enter_context(tc.tile_pool(name="wps", bufs=1, space="PSUM"))

        # --- PE warm-up: dense dummy matmuls while the first DMAs fly ---
        wu_w = const.tile([128, 128], FP16, name="wu_w")
        wu_x = const.tile([128, 128], FP16, name="wu_x")
        nc.gpsimd.memset(wu_w[:], 0.0)
        nc.gpsimd.memset(wu_x[:], 0.0)
        wu_ps = wps.tile([128, 128], F32, name="wu_ps")
        for _ in range(WARMUP):
            nc.tensor.matmul(wu_ps[:], wu_w[:], wu_x[:], start=True, stop=True)

        # b1 on the (otherwise idle) gpsimd SWDGE ring, ahead of the junk
        # store so the warm-up dependency chain can't delay it
        b1s = const.tile([128, E * HCL], F32, name="b1s")
        nc.gpsimd.dma_start(b1s[:], b1_d[:])

        # give the warm-up psum a reader chain ending in DRAM so every tile
        # has readers (junk output, never consumed by the host)
        wu_sink = const.tile([128, 4], F32, name="wu_sink")
        nc.vector.tensor_copy(wu_sink[:], wu_ps[:, :4])
        nc.gpsimd.dma_start(junk_d[:], wu_sink[:])

        w1t = {}
        w2t = {}

        def load_weights(e, first):
            w1e = w1p.tile([128, HCL, DC, 128], FP16, name=f"w1e{e}", tag="w1")
            w2e = w2p.tile([128, HCL, DC, 128], FP16, name=f"w2e{e}", tag="w2")
            if first:
                # split fine so the first matmuls' weights land ASAP; the
                # first W2 rides the sync ring (idle after g0's small xg)
                # so both HWDGE rings pull weights in parallel at the head
                for hc in range(0, HCL, 2):
                    nc.scalar.dma_start(w1e[:, hc : hc + 2], w1_d[e, :, hc : hc + 2])
                for hc in range(0, HCL, 2):
                    nc.sync.dma_start(w2e[:, hc : hc + 2], w2_d[e, :, hc : hc + 2])
            else:
                nc.scalar.dma_start(w1e[:], w1_d[e])
                nc.scalar.dma_start(w2e[:], w2_d[e])
            w1t[e] = w1e
            w2t[e] = w2e

        # expert blocks: [(e, [(off, L), ...]), ...] in group order
        blocks = []
        off = 0
        for e, L in groups:
            if blocks and blocks[-1][0] == e:
                blocks[-1][1].append((off, L))
            else:
                blocks.append((e, [(off, L)]))
            off += L

        for bi, (e, glist) in enumerate(blocks):
            firstblk = bi == 0
            lastblk = bi == len(blocks) - 1
            bias_col = e * HCL

            # --- input DMAs (prefetch horizon = pool bufs) ---
            xts = []
            for j, (off, L) in enumerate(glist):
                xg = xgp.tile([128, DC, L], FP16, name=f"xg{off}", tag="xg")
                if firstblk and j == 0:
                    # fine-grained so the very first matmuls start early
                    for dc in range(0, DC, 2):
                        nc.sync.dma_start(
                            xg[:, dc : dc + 2, :],
                            xg_d[:, DC * off + dc * L : DC * off + (dc + 2) * L],
                        )
                else:
                    nc.sync.dma_start(
                        xg[:], xg_d[:, DC * off : DC * off + DC * L]
                    )
                xts.append(xg)
            load_weights(e, first=firstblk)
            w1e, w2e = w1t[e], w2t[e]

            # --- P1 for every group of this expert (only W1 needed) ---
            hts = []
            for j, (off, L) in enumerate(glist):
                xg = xts[j]
                ht = htp.tile([128, HCL, L], FP16, name=f"ht{off}", tag="ht")
                if firstblk:
                    # head: accumulate two hc psums side by side so xg is
                    # consumed dc-piece by dc-piece as the DMAs land, and
                    # W1 hc-pair by hc-pair -- demand tracks supply
                    for hc0 in range(0, HCL, 2):
                        pss = [
                            psp.tile([128, L], F32, name=f"p1_{off}_{hc}", tag="ps")
                            for hc in (hc0, hc0 + 1)
                        ]
                        for dc in range(DC):
                            for k, hc in enumerate((hc0, hc0 + 1)):
                                nc.tensor.matmul(
                                    pss[k][:],
                                    w1e[:, hc, dc, :],
                                    xg[:, dc, :],
                                    start=(dc == 0),
                                    stop=(dc == DC - 1),
                                )
                        for k, hc in enumerate((hc0, hc0 + 1)):
                            nc.scalar.activation(
                                ht[:, hc, :],
                                pss[k][:],
                                mybir.ActivationFunctionType.Relu,
                                bias=b1s[:, bias_col + hc : bias_col + hc + 1],
                            )
                    hts.append(ht)
                    continue
                for hc in range(HCL):
                    # split the last hc's psum so its epilogue overlaps the
                    # next matmuls (shortens the P1->P2 handoff)
                    if hc == HCL - 1 and L >= 64:
                        h1 = (L // 2 + 31) // 32 * 32
                        parts = [(0, h1), (h1, L - h1)]
                    else:
                        parts = [(0, L)]
                    for c0, cn in parts:
                        ps = psp.tile(
                            [128, cn], F32, name=f"p1_{off}_{hc}_{c0}", tag="ps"
                        )
                        for dc in range(DC):
                            nc.tensor.matmul(
                                ps[:],
                                w1e[:, hc, dc, :],
                                xg[:, dc, c0 : c0 + cn],
                                start=(dc == 0),
                                stop=(dc == DC - 1),
                            )
                        nc.scalar.activation(
                            ht[:, hc, c0 : c0 + cn],
                            ps[:],
                            mybir.ActivationFunctionType.Relu,
                            bias=b1s[:, bias_col + hc : bias_col + hc + 1],
                        )
                hts.append(ht)

            # --- P2 + store for every group (only W2 needed) ---
            for j, (off, L) in enumerate(glist):
                ht = hts[j]
                lastgrp = lastblk and j == len(glist) - 1
                ot = outp.tile([128, DC, L], FP16, name=f"ot{off}", tag="ot")
                for dc in range(DC):
                    if lastgrp and dc == DC - 1 and L >= 64:
                        h1 = (L // 2 + 31) // 32 * 32
                        parts = [(0, h1), (h1, L - h1)]
                    else:
                        parts = [(0, L)]
                    for c0, cn in parts:
                        ps = psp.tile(
                            [128, cn], F32, name=f"p2_{off}_{dc}_{c0}", tag="ps"
                        )
                        for hc in range(HCL):
                            nc.tensor.matmul(
                                ps[:],
                                w2e[:, hc, dc, :],
                                ht[:, hc, c0 : c0 + cn],
                                start=(hc == 0),
                                stop=(hc == HCL - 1),
                            )
                        nc.vector.tensor_copy(ot[:, dc, c0 : c0 + cn], ps[:])
                # the last two groups store per-dcpair, issued as each psum
                # pair drains so the transfers hide behind the remaining
                # matmuls (the very last on both HWDGE rings for minimal
                # completion latency); earlier groups use one big-line DMA
                neartail = lastblk and j >= len(glist) - 2
                if neartail:
                    engs = (
                        [nc.sync, nc.scalar, nc.sync, nc.scalar]
                        if lastgrp
                        else [nc.gpsimd] * 4
                    )
                    for i, dc in enumerate(range(0, DC, 2)):
                        engs[i].dma_start(
                            out_d[:, DC * off + dc * L : DC * off + (dc + 2) * L],
                            ot[:, dc : dc + 2, :],
                        )
                else:
                    nc.gpsimd.dma_start(
                        out_d[:, DC * off : DC * off + DC * L], ot[:]
                    )

    nc.compile()
    return nc


def _route(xs, Wg, k):
    """Top-k routing + softmax combine weights, mirroring jax.lax.top_k
    (descending, ties broken by lower index) + softmax over the k logits."""
    router = xs @ Wg.T  # (T, E) fp32
    t = np.arange(xs.shape[0])[:, None]
    sel = np.zeros((xs.shape[0], k), np.int64)
    masked = router.copy()
    for j in range(k):
        sel[:, j] = np.argmax(masked, axis=1)
        masked[t[:, 0], sel[:, j]] = -np.inf
    logits = router[t, sel]  # (T, k), descending
    ex = np.exp((logits - logits[:, :1]).astype(np.float32))
    wgt = (ex / ex.sum(axis=1, keepdims=True)).astype(np.float32)
    return sel, wgt


def _run(inputs, trace=False, **rk):
    xs = np.asarray(inputs["xs"], np.float32)
    top_k = int(inputs["top_k"])
    Wg = np.asarray(inputs["Wg"], np.float32)
    W1 = np.asarray(inputs["W1"], np.float32)
    b1 = np.asarray(inputs["b1"], np.float32)
    W2 = np.asarray(inputs["W2"], np.float32)
    b2 = np.asarray(inputs["b2"], np.float32)

    sel, wgt = _route(xs, Wg, top_k)  # (T, k)
    P = T * top_k
    flat_sel = sel.ravel()
    counts = np.bincount(flat_sel, minlength=E)
    groups = _plan_groups(counts)
    # pair list in the group plan's expert order (token order within expert)
    eorder = []
    for e, _ in groups:
        if e not in eorder:
            eorder.append(e)
    order = np.concatenate([np.nonzero(flat_sel == e)[0] for e in eorder])
    pair_tok = order // top_k
    pair_wgt = wgt.ravel()[order]

    key = (groups,)
    if key not in _prog_cache:
        _prog_cache[key] = _build_program(groups)
    nc = _prog_cache[key]

    # --- host-side packing ---
    # xg: [128, DC*P] fp16; group block layout [r][dc][c]
    xs16 = xs.astype(np.float16)
    xg = np.empty((128, DC * P), np.float16)
    off = 0
    for e, L in groups:
        blk = xs16[pair_tok[off : off + L]]  # [L, 1024]
        # [L, D] -> [dc, r, c] -> [r, dc, c]
        xg[:, DC * off : DC * (off + L)] = (
            blk.T.reshape(DC, 128, L).transpose(1, 0, 2).reshape(128, DC * L)
        )
        off += L

    # weights: full transposes once, then per-core slices
    # W1 [E, D, H] -> [E, hc(32), r(128), dc(8), h(128)]  ([e][hc][r][dc][h])
    w1_all = np.ascontiguousarray(
        W1.reshape(E, DC, 128, H // 128, 128).transpose(0, 3, 2, 1, 4)
    ).astype(np.float16)
    # W2 [E, H, D] -> [E, hc(32), r(128), dc(8), d(128)] (natural reshape)
    w2_all = W2.reshape(E, H // 128, 128, DC, 128).astype(np.float16)
    b1_all = b1.reshape(E, H // 128, 128)  # [e][hc][r]

    in_maps = []
    for m in range(NCORES):
        hsl = slice(HCL * m, HCL * (m + 1))
        w1m = np.ascontiguousarray(w1_all[:, hsl].transpose(0, 2, 1, 3, 4))
        w2m = np.ascontiguousarray(w2_all[:, hsl].transpose(0, 2, 1, 3, 4))
        b1m = np.ascontiguousarray(
            b1_all[:, hsl].transpose(2, 0, 1).reshape(128, E * HCL)
        ).astype(np.float32)
        in_maps.append({"xg": xg, "w1": w1m, "w2": w2m, "b1": b1m})

    res = run_bass_kernel_spmd(
        nc, in_maps, core_ids=list(range(NCORES)), trace=trace, **rk
    )

    # --- host-side reduce + combine + scatter ---
    ysum = np.zeros((D, P), np.float32)
    for m in range(NCORES):
        o = res.results[m]["out"]  # [128, DC*P] fp16
        off = 0
        for e, L in groups:
            blk = o[:, DC * off : DC * (off + L)].reshape(128, DC, L)
            ysum[:, off : off + L] += blk.transpose(1, 0, 2).reshape(D, L)
            off += L
    ysum *= pair_wgt[None, :]

    # gather back to tokens: position of each (t, j) pair in the sorted order
    pos = np.empty(P, np.int64)
    pos[order] = np.arange(P)
    pos = pos.reshape(T, top_k)
    out = ysum.T[pos[:, 0]].copy()
    for j in range(1, top_k):
        out += ysum.T[pos[:, j]]
    # b2 applied once, weighted by the combine weights: out += comb @ b2
    comb = np.zeros((T, E), np.float32)
    np.put_along_axis(comb, sel, wgt, axis=1)
    out += comb @ b2
    return out, res


def kernel(**inputs) -> np.ndarray:
    out, _ = _run(inputs)
    return out
